# revision 34
# baseline (speedup 1.0000x reference)
"""Causal attention (B=8, S=2048, D=128, f32) on 8 TRN2 NeuronCores.

Strategy: batch-parallel SPMD - each core computes full causal attention for
one batch element.

Per-core algorithm (layouts chosen so softmax/PV need no on-chip transposes):
  - Host passes Q^T, K^T as [D=128, S=2048] bf16 (D on partitions) and V
    pre-arranged as VS [128, S] bf16 where column block j holds V rows
    [128j, 128j+128).
  - Scores are computed transposed, per key block j:
        S^T_j[k, q] = (K^T_j)-stationary.T @ Q^T-moving   (PSUM, f32)
  - Causal mask applied multiplicatively on the bf16 P^T tile of the
    diagonal block (VectorE, [128,128] per block; DVE has slack). An
    additive in-PSUM mask matmul was tried and reverted: Tile does not
    order accumulating matmuls against each other (WAW on PSUM is treated
    as commutative), so exp raced the score matmul.
  - exp with the 1/sqrt(D) scale folded into ScalarE's activation affine,
    PSUM -> SBUF, output in bf16 (P^T tiles). ScalarE is the critical
    engine (~17.5us busy) - everything else is scheduled to keep it fed.
  - Z[k,q] = sum_j P^T_j accumulated on VectorE (bf16 tensor_tensor, 2x
    mode). Block 0 of each pass writes its exp output directly into Z
    (saves a copy). Rowsum is then ONE 512-col ones-stationary matmul per
    512-wide q chunk (PE cost 2048 cols total instead of 17408 for
    per-block rowsum matmuls - saves ~6.4us of PE).
  - out^T[d, q] += V_j-stationary @ P^T_j-moving (bf16 in, f32 accumulate).
  - Normalize per 512-wide q-chunk as soon as its accumulation finishes:
    reciprocal_approx_fast on the replicated rowsum, multiply, write bf16,
    DMA out (output is bf16; host upcasts - halves store traffic, rel-err
    cost ~0.4% against a 2% budget). Pass-0 chunks are evacuated from
    PSUM with a fast copy first so pass-1's first PV matmuls (WAR on
    out^T PSUM) don't wait for the recip+mul chain.
  - Final chunk (q [1536,2048)) drains fast: its rowsum accumulates
    incrementally on the PE (ones @ Z_partial after block 11, ones @ P^T_j
    for blocks 12-15, whose Z-adds are skipped), and recip/mul/store run
    per 256-col half as each half's rowsum becomes final - the first half
    completes during the last two exps.
  - The two narrowest tail block pairs of pass 1 are packed into shared
    score tiles so one exp covers both (fewer ScalarE call bubbles).
  - Constants (ones / causal-mask) are generated on-chip with memset +
    affine_select instead of DMAs; input DMAs are paced to consumption
    (flooding the queues saturates HBM across the 8 cores and starves the
    critical path; DMA completion latency is ~2us).
  - Host transposes out^T back to [S, D].

TensorE work is software-pipelined: scores for key block j+2 are emitted
before PV of block j so the PE never head-of-line blocks on ScalarE's exp.
The q axis is processed in two passes of 1024 so PSUM fits:
  staging S^T [128,1024] x2 bufs (4 banks) + out^T [128,1024] (2 banks)
  + 2x rowsum [128,512] (2 banks) = 8 banks.
"""

import math
import sys

import numpy as np
import ml_dtypes

sys.path.insert(0, "/opt/trn_rl_repo")

from concourse import bacc, mybir
from concourse.bass_utils import run_bass_kernel_spmd
from concourse.tile import TileContext

F32 = mybir.dt.float32
BF16 = mybir.dt.bfloat16
BF16_NP = np.dtype(ml_dtypes.bfloat16)

B, S, D = 8, 2048, 128
NBLK = S // 128  # 16 key blocks
HALF = 1024  # q-pass width
SCALE = 1.0 / math.sqrt(D)
MASKNEG = -1e9

_NC_CACHE = None


def _chunks_for_block(j, q0):
    """Matmul chunks for key block j in pass [q0, q0+HALF): list of
    (a, b, h) global q ranges clipped to psum bank h."""
    k0 = 128 * j
    q_lo = max(q0, k0)
    out = []
    for h in range(2):
        a = max(q_lo, q0 + 512 * h)
        b = q0 + 512 * (h + 1)
        if a < b:
            out.append((a, b, h))
    return out


def _build_nc():
    nc = bacc.Bacc("TRN2", target_bir_lowering=False, debug=False, num_devices=8)

    qt_d = nc.dram_tensor("QT", [D, S], BF16, kind="ExternalInput")
    kt_d = nc.dram_tensor("KT", [D, S], BF16, kind="ExternalInput")
    vs_d = nc.dram_tensor("VS", [128, S], BF16, kind="ExternalInput")
    out_d = nc.dram_tensor("out", [D, S], BF16, kind="ExternalOutput")

    with TileContext(nc) as tc:
        with (
            tc.tile_pool(name="persist", bufs=1) as persist,
            tc.tile_pool(name="ptp", bufs=4) as ptp,
            tc.tile_pool(name="zp", bufs=2) as zp,
            tc.tile_pool(name="epi", bufs=4) as epi,
            tc.tile_pool(name="spool", bufs=2, space="PSUM") as spool,
            tc.tile_pool(name="opool", bufs=1, space="PSUM") as opool,
            tc.tile_pool(name="rpool", bufs=2, space="PSUM") as rpool,
        ):
            qt = persist.tile([D, S], BF16, tag="qt")
            kt = persist.tile([D, S], BF16, tag="kt")
            vs = persist.tile([128, S], BF16, tag="vs")  # col block j = V rows

            # warm the PE clock (HAM un-throttles after ~3.4us of
            # sustained activity) with dummy matmuls on scratch data while
            # input DMAs are in flight; results are never read. The PE is
            # in-order, so the warm stream must END when the first score
            # inputs land (~1.7us after engines start) - 3 cold matmuls.
            # memsets run on VectorE (idle) so GpSimd's serial software-DGE
            # queue can issue the critical input DMAs immediately.
            pe_scr = persist.tile([128, 512], BF16, tag="pe_scr")
            nc.vector.memset(pe_scr[:, :], 1.0)
            warm_ps = spool.tile([128, HALF], F32, tag="sps", name="warm_ps")
            for _w in range(3):
                nc.tensor.matmul(
                    warm_ps[:, 0:512],
                    pe_scr[:, 0:128],
                    pe_scr[:, :],
                    start=True,
                    stop=True,
                )

            # ---- input DMAs ----
            # The load schedule must be paced to consumption: flooding the
            # HW queue with large chunks saturates HBM (8 cores pull
            # simultaneously) and starves the critical-path transfers.
            # The critical first chunks ride the SP hardware queue (first
            # issue fires ~0.7us before the Pool software-DGE's). Pool
            # carries the second tier + all of vs as a naturally
            # rate-limited trickle.
            nc.gpsimd.dma_start(vs[:, 0:256], vs_d[:, 0:256])

            # on-chip constants (no DMAs):
            #   mask[k, q]  = 1 where q >= k else 0      (causal, bf16)
            #   ones_b      = 1                          (rowsum stationary)
            mask = persist.tile([128, 128], BF16, tag="mask")
            nc.gpsimd.memset(mask[:, :], 1.0)
            nc.gpsimd.affine_select(
                out=mask[:, :],
                in_=mask[:, :],
                compare_op=mybir.AluOpType.is_ge,
                fill=0.0,
                base=0,
                pattern=[[1, 128]],
                channel_multiplier=-1,
            )
            ones_b = persist.tile([128, 128], BF16, tag="ones_b")
            nc.vector.memset(ones_b[:, :], 1.0)

            # warm the ScalarE exp table while input DMAs run
            warm_src = persist.tile([1, 16], F32, tag="warm_src")
            nc.vector.memset(warm_src[:, :], 0.0)
            warm = epi.tile([1, 16], F32, tag="warm")
            nc.scalar.activation(
                warm[:, :],
                warm_src[:, :],
                mybir.ActivationFunctionType.Exp,
                scale=SCALE,
            )

            # rest of vs on the Pool trickle (block j needed at PV_j,
            # paced by the exp stream at ~1.1us/block)
            for c in range(1, 8):
                nc.gpsimd.dma_start(vs[:, c * 256 : (c + 1) * 256],
                                    vs_d[:, c * 256 : (c + 1) * 256])

            # ---- bulk input DMAs on the SP hardware queue ----
            # The first exp's coalesced wait runs through scores block 1,
            # so qt[0:512] and kt[0:256] ride the first two issues - one
            # completion semaphore each covers everything blocks 0-1 need.
            nc.sync.dma_start(qt[:, 0:512], qt_d[:, 0:512])
            nc.sync.dma_start(kt[:, 0:256], kt_d[:, 0:256])
            nc.sync.dma_start(qt[:, 512:1024], qt_d[:, 512:1024])
            nc.sync.dma_start(kt[:, 256:512], kt_d[:, 256:512])
            nc.sync.dma_start(kt[:, 512:1024], kt_d[:, 512:1024])
            nc.sync.dma_start(qt[:, 1024:1280], qt_d[:, 1024:1280])
            nc.sync.dma_start(qt[:, 1280:2048], qt_d[:, 1280:2048])
            nc.sync.dma_start(kt[:, 1024:1536], kt_d[:, 1024:1536])
            nc.sync.dma_start(kt[:, 1536:2048], kt_d[:, 1536:2048])

            pts = {}
            zs = {}
            # Narrow tail blocks are packed in PAIRS into one sps tile so
            # a single exp instruction covers both (saves the ~260ns
            # ScalarE per-call bubble 4x and shortens the drain). The map
            # gives the pair-FIRST block's tile column offset; its partner
            # (j+1) lands at offset 0 and the placements tile exactly.
            PAIR_FIRST = {(1, 12): 384, (1, 14): 128}
            sps_shared = {}

            def emit_scores(qh, j):
                """QK^T chunks + per-block exp + diag mask for key block j.
                Pass-independent so the two-deep score pipeline can run
                across the pass boundary. Block 0's exp writes the pass Z
                accumulator directly. pts[(qh, j)] = (pt_tile, col_ofs)."""
                q0 = qh * HALF
                k0 = 128 * j
                a0 = max(q0, k0)
                w = q0 + HALF - a0
                key = (qh, j)
                pair_second = (qh, j - 1) in PAIR_FIRST
                if pair_second:
                    sps = sps_shared.pop((qh, j - 1))
                    ofs = 0
                else:
                    sps = spool.tile([128, HALF], F32, tag="sps",
                                     name=f"sps_{qh}_{j}")
                    ofs = PAIR_FIRST.get(key, a0 - q0)
                    if key in PAIR_FIRST:
                        sps_shared[key] = sps
                # score matmuls over tile cols [ofs, ofs+w), split at PSUM
                # bank boundaries
                c = ofs
                while c < ofs + w:
                    ce = min(ofs + w, (c // 512 + 1) * 512)
                    qa = a0 + (c - ofs)
                    qb = a0 + (ce - ofs)
                    if qh == 0 and j == 0 and c == 0:
                        nc.tensor.matmul(sps[:, 0:256], kt[:, 0:128],
                                         qt[:, 0:256], start=True,
                                         stop=True)
                        nc.tensor.matmul(sps[:, 256:512], kt[:, 0:128],
                                         qt[:, 256:512], start=True,
                                         stop=True)
                    else:
                        nc.tensor.matmul(
                            sps[:, c:ce],
                            kt[:, k0 : k0 + 128],
                            qt[:, qa:qb],
                            start=True,
                            stop=True,
                        )
                    c = ce
                if key in PAIR_FIRST:
                    return  # exp deferred to the pair's second block
                if pair_second:
                    ofs1 = PAIR_FIRST[(qh, j - 1)]
                    end = ofs1 + (q0 + HALF - 128 * (j - 1))
                    pt = ptp.tile([128, HALF], BF16, tag="pt",
                                  name=f"pt_{qh}_{j}")
                    nc.scalar.activation(
                        pt[:, 0:end], sps[:, 0:end],
                        mybir.ActivationFunctionType.Exp, scale=SCALE,
                    )
                    # both pair members are diagonal blocks
                    nc.vector.tensor_mul(pt[:, ofs1 : ofs1 + 128],
                                         pt[:, ofs1 : ofs1 + 128],
                                         mask[:, :])
                    nc.vector.tensor_mul(pt[:, 0:128], pt[:, 0:128],
                                         mask[:, :])
                    pts[(qh, j - 1)] = (pt, ofs1)
                    pts[key] = (pt, 0)
                    return
                if j == 0:
                    pt = zp.tile([128, HALF], BF16, tag="z",
                                 name=f"z_{qh}")
                    zs[qh] = pt
                else:
                    pt = ptp.tile([128, HALF], BF16, tag="pt",
                                  name=f"pt_{qh}_{j}")
                if j == 0 and qh == 0:
                    # split so the first exp only needs qt[0:512] loaded
                    nc.scalar.activation(
                        pt[:, 0:512], sps[:, 0:512],
                        mybir.ActivationFunctionType.Exp, scale=SCALE,
                    )
                    nc.scalar.activation(
                        pt[:, 512:HALF], sps[:, 512:HALF],
                        mybir.ActivationFunctionType.Exp, scale=SCALE,
                    )
                else:
                    nc.scalar.activation(
                        pt[:, ofs : ofs + w],
                        sps[:, ofs : ofs + w],
                        mybir.ActivationFunctionType.Exp,
                        scale=SCALE,
                    )
                if k0 >= q0:
                    nc.vector.tensor_mul(
                        pt[:, ofs : ofs + 128],
                        pt[:, ofs : ofs + 128],
                        mask[:, :],
                    )
                pts[key] = (pt, ofs)

            emit_scores(0, 0)
            emit_scores(0, 1)

            for qh in range(2):
                q0 = qh * HALF  # global q offset of this pass
                njb = (q0 + HALF) // 128  # key blocks this pass

                out_ps = opool.tile([D, HALF], F32, tag="outps",
                                    name=f"outps_{qh}")
                # last key block that touches each 512-half of out_ps
                j_last = [min(njb - 1, (q0 + 512 * (h + 1)) // 128 - 1)
                          for h in range(2)]

                def emit_consume(j, pt, ofs, qh=qh, q0=q0, skip_z=False):
                    """Z accumulation (DVE) + PV (PE) for key block j.
                    ofs = column offset of this block's q-range start in
                    the pt tile."""
                    k0 = 128 * j
                    chunks = _chunks_for_block(j, q0)
                    a0 = max(q0, k0)
                    if j > 0 and not skip_z:
                        z = zs[qh]
                        nc.vector.tensor_add(
                            z[:, a0 - q0 : HALF],
                            z[:, a0 - q0 : HALF],
                            pt[:, ofs : ofs + (HALF - (a0 - q0))],
                        )
                    for (a, b, h) in chunks:
                        nc.tensor.matmul(
                            out_ps[:, a - q0 : b - q0],
                            vs[:, k0 : k0 + 128],
                            pt[:, ofs + (a - a0) : ofs + (b - a0)],
                            start=(j == 0),
                            stop=(j == j_last[h]),
                        )

                def emit_epi_half(h, qh=qh, q0=q0):
                    """Rowsum + normalize + store q-chunk
                    [q0+512h, q0+512h+512)."""
                    sl = slice(512 * h, 512 * (h + 1))
                    z = zs[qh]
                    rs = rpool.tile([128, 512], F32, tag="rs",
                                    name=f"rs_{qh}_{h}")
                    nc.tensor.matmul(rs[:, :], ones_b[:, :], z[:, sl],
                                     start=True, stop=True)
                    o_fin = epi.tile([D, 512], BF16, tag="o_fin",
                                     name=f"ofin_{qh}_{h}")
                    if qh == 0:
                        # evacuate the PSUM chunk with a fast copy FIRST so
                        # pass-1's first PV matmuls (WAR on out_ps) don't
                        # wait for the recip+mul chain
                        o_evac = epi.tile([D, 512], F32, tag="o_evac",
                                          name=f"oev_{qh}_{h}")
                        nc.vector.tensor_copy(o_evac[:, :], out_ps[:, sl])
                    rb = epi.tile([128, 512], F32, tag="rb",
                                  name=f"rb_{qh}_{h}")
                    nc.vector.reciprocal_approx_fast(
                        out=rb[:, :], in_=rs[:, :]
                    )
                    if qh == 0:
                        nc.vector.tensor_mul(o_fin[:, :], o_evac[:, :],
                                             rb[:, :])
                    else:
                        nc.vector.tensor_mul(o_fin[:, :], out_ps[:, sl],
                                             rb[:, :])
                    nc.sync.dma_start(
                        out_d[:, q0 + 512 * h : q0 + 512 * (h + 1)],
                        o_fin[:, :],
                    )

                # software pipeline, two deep on the consume side and
                # continued ACROSS the pass boundary: pass-1's first two
                # score groups are emitted during pass-0's last iterations.
                # Final chunk (pass 1, q [1536,2048)) accumulates its rowsum
                # incrementally on the PE (ones @ Z_partial after block 11,
                # then ones @ pt_j for blocks 12-15, whose Z-adds are
                # skipped) so the drain after the last exp is just
                # recip+mul+store.
                rs11 = rb11 = of11 = None
                for j in range(njb):
                    nj = j + 2
                    if nj < njb:
                        emit_scores(qh, nj)
                    elif qh == 0:
                        emit_scores(1, nj - njb)
                    pt, ofs = pts.pop((qh, j))
                    final_rs = qh == 1 and j >= 12
                    emit_consume(j, pt, ofs, skip_z=final_rs)
                    if final_rs:
                        a0 = 128 * j
                        nc.tensor.matmul(
                            rs11[:, a0 - (q0 + 512) : 512],
                            ones_b[:, :],
                            pt[:, ofs : ofs + (2048 - a0)],
                            start=False,
                            stop=(j == 15),
                        )
                    if qh == 1 and j == 11:
                        rs11 = rpool.tile([128, 512], F32, tag="rs",
                                          name="rs_1_1")
                        nc.tensor.matmul(rs11[:, :], ones_b[:, :],
                                         zs[1][:, 512:HALF],
                                         start=True, stop=False)
                    if j == j_last[0] and not (qh == 1):
                        emit_epi_half(0)
                    if j == j_last[1] and qh == 0:
                        emit_epi_half(1)
                    if qh == 1 and j == j_last[0]:
                        emit_epi_half(0)
                    if qh == 1 and j in (13, 15):
                        # pipelined final-chunk epilogue: each 256-col half
                        # of rowsum [1536:2048) is final once blocks <= 13
                        # (resp. 15) have accumulated, so the first half's
                        # recip/mul/store runs during the last two exps and
                        # the post-exp drain handles only 256 cols
                        c = 0 if j == 13 else 1
                        cs = slice(256 * c, 256 * (c + 1))
                        if c == 0:
                            rb11 = epi.tile([128, 512], F32, tag="rb",
                                            name="rb_1_1")
                            of11 = epi.tile([D, 512], BF16, tag="o_fin",
                                            name="ofin_1_1")
                        nc.vector.reciprocal_approx_fast(
                            out=rb11[:, cs], in_=rs11[:, cs]
                        )
                        nc.vector.tensor_mul(
                            of11[:, cs],
                            out_ps[:, 512 + 256 * c : 512 + 256 * (c + 1)],
                            rb11[:, cs],
                        )
                        eng = nc.sync if c == 0 else nc.gpsimd
                        eng.dma_start(
                            out_d[:, 1536 + 256 * c : 1536 + 256 * (c + 1)],
                            of11[:, cs],
                        )

    nc.compile()
    return nc


def _get_nc():
    global _NC_CACHE
    if _NC_CACHE is None:
        _NC_CACHE = _build_nc()
    return _NC_CACHE


def _in_maps(Q, K, V):
    maps = []
    for b in range(B):
        vsb = np.ascontiguousarray(
            V[b].reshape(NBLK, 128, D).transpose(1, 0, 2).reshape(128, S)
        ).astype(BF16_NP)
        maps.append(
            {
                "QT": np.ascontiguousarray(Q[b].T).astype(BF16_NP),
                "KT": np.ascontiguousarray(K[b].T).astype(BF16_NP),
                "VS": vsb,
            }
        )
    return maps


def kernel(Q, K, V):
    Q = np.asarray(Q, dtype=np.float32)
    K = np.asarray(K, dtype=np.float32)
    V = np.asarray(V, dtype=np.float32)
    assert Q.shape == (B, S, D), Q.shape

    nc = _get_nc()
    res = run_bass_kernel_spmd(nc, _in_maps(Q, K, V), core_ids=list(range(B)))
    return np.stack(
        [np.ascontiguousarray(res.results[b]["out"].T).astype(np.float32)
         for b in range(B)],
        axis=0,
    )
